# revision 19
# baseline (speedup 1.0000x reference)
"""Trainium2 Bass kernel for MedSegNetV2 GLCM-feature martingale.

Math (K=3 window, THETA=1, per pixel over zero-padded 3x3 neighborhood):
  m   = mean(win)                 (box9 / 9)
  M2  = mean(win^2)
  d   = M2 - m^2                  (biased variance, >= 0 up to fp error)
  contrast = d / max(1.125*d, 1e-6)        == min(8/9, 1e6*d)  exactly
  energy   = M2
  entropy  = -mean(t*ln t), t = max(win, 1e-6)
  homog    = 1 / (1 + mean|win - m| + 1e-6)
  out_f    = clip(max(f,1e-5)*exp(-.5), 1e-4, 1e4)
           == max(beta*f, 1e-4) for these value ranges (upper clip can't bind:
              energy<=~30, contrast<=8/9, |entropy|<=~10, homog<=1)

Sharding: pure data parallel, core k processes batch k (64 channel slices).
Layout per core: 16 groups of 4 slices; per group 2 row-bands of 112 rows;
x tile [114 part = 112 rows + 2 halo, 908 = 4*(224+2 pad cols) + 4 spare].
Vertical 3-sums via TensorE banded matmuls; horizontal 3-sums via shifted
col accesses (DVE adds for x, shifted-rhs accumulating matmuls for sq/e).
|win - m| for the 9 offsets: 9 tensor_tensor subs (DVE/GPSIMD split) into an
interleaved stack, then one DVE tensor_reduce(add, apply_absolute_value).
1/z computed as exp(ln(beta) - ln(z)) on ACT (no reciprocal needed).
"""

import math
from contextlib import ExitStack

import numpy as np

import concourse.bass as bass
import concourse.bacc as bacc
import concourse.tile as tile
from concourse import mybir
from concourse.bass_utils import run_bass_kernel_spmd

F32 = mybir.dt.float32
AF = mybir.ActivationFunctionType
OP = mybir.AluOpType

B, C, H, W = 8, 64, 224, 224
NCORES = 8
BETA = math.exp(-0.5)
LNBETA = -0.5

GROUPS = 16          # groups of 4 slices per core
UNITS = 4            # slices per group
UCOL = 226           # per-slice unit width: [pad, 224 data, pad]
XW = UNITS * UCOL + 4  # 908, even, includes 4 zero spare cols
BAND = 112           # output rows per band
KP = BAND + 2        # input partitions incl halo rows
CHUNK = 452          # output cols per chunk (= 2 units)

# which of the 9 (dy,dx) subtractions run on DVE vs GPSIMD
DVE_SUBS = (0, 2, 4, 6)


def _banded(val: float) -> np.ndarray:
    w = np.zeros((KP, BAND), dtype=np.float32)
    for p in range(BAND):
        for k in (p, p + 1, p + 2):
            w[k, p] = val
    return w


def _build(groups=GROUPS, num_devices=NCORES,
           use_reduce=True, use_ttr=False, use_gp=True):
    nslice = groups * UNITS
    nc = bacc.Bacc("TRN2", target_bir_lowering=False, debug=False,
                   num_devices=num_devices)
    x_in = nc.dram_tensor("x", [nslice, H, W], F32, kind="ExternalInput")
    w_box_d = nc.dram_tensor("w_box", [KP, BAND], F32, kind="ExternalInput")
    w_boxn_d = nc.dram_tensor("w_boxn", [KP, BAND], F32, kind="ExternalInput")
    out_d = nc.dram_tensor("out", [nslice * 4, H, W], F32,
                           kind="ExternalOutput")

    with tile.TileContext(nc) as tc, ExitStack() as ctx:
        consts = ctx.enter_context(tc.tile_pool(name="consts", bufs=1))
        io = ctx.enter_context(tc.tile_pool(name="io", bufs=2))
        mid = ctx.enter_context(tc.tile_pool(name="mid", bufs=2))
        dpool = ctx.enter_context(tc.tile_pool(name="dpool", bufs=2))
        small = ctx.enter_context(tc.tile_pool(name="small", bufs=3))
        outp = ctx.enter_context(tc.tile_pool(name="outp", bufs=3))
        psum = ctx.enter_context(tc.tile_pool(name="psum", bufs=2, space="PSUM"))

        w_box = consts.tile([KP, BAND], F32)
        w_boxn = consts.tile([KP, BAND], F32)
        nc.sync.dma_start(out=w_box[:], in_=w_box_d[:])
        nc.sync.dma_start(out=w_boxn[:], in_=w_boxn_d[:])
        b_z = consts.tile([BAND, 1], F32)
        nc.vector.memset(b_z[:], 1.0 + 1e-6)
        b_lnb = consts.tile([BAND, 1], F32)
        nc.vector.memset(b_lnb[:], LNBETA)

        for g in range(groups):
            for half in range(2):
                r0 = half * BAND
                x_t = io.tile([KP, XW], F32)
                nc.gpsimd.memset(x_t[:], 0.0)
                for u in range(UNITS):
                    s = g * UNITS + u
                    c0 = UCOL * u + 1
                    if half == 0:
                        nc.sync.dma_start(out=x_t[1:KP, c0:c0 + W],
                                          in_=x_in[s, 0:KP - 1, :])
                    else:
                        nc.sync.dma_start(out=x_t[0:KP - 1, c0:c0 + W],
                                          in_=x_in[s, r0 - 1:H, :])

                # engines can't read SBUF at partition offsets 1/2 — make
                # row-shifted copies via DMA (partition shifts legal there)
                x_mid = io.tile([BAND, XW], F32)
                nc.sync.dma_start(out=x_mid[:], in_=x_t[1:1 + BAND, :])
                x_dn = io.tile([BAND, XW], F32)
                nc.sync.dma_start(out=x_dn[:], in_=x_t[2:2 + BAND, :])

                tcl = mid.tile([KP, XW], F32)
                nc.vector.tensor_scalar_max(tcl[:], x_t[:], 1e-6)
                lnt = mid.tile([KP, XW], F32)
                nc.scalar.activation(lnt[:], tcl[:], AF.Ln)
                e_t = mid.tile([KP, XW], F32)
                (nc.gpsimd if use_gp else nc.vector).tensor_tensor(
                    out=e_t[:], in0=tcl[:], in1=lnt[:], op=OP.mult)
                sq = mid.tile([KP, XW], F32)
                nc.scalar.activation(sq[:], x_t[:], AF.Square)
                hxa = mid.tile([KP, XW - 4], F32)
                nc.vector.tensor_tensor(out=hxa[:], in0=x_t[:, 0:XW - 4],
                                        in1=x_t[:, 1:XW - 3], op=OP.add)
                hx = mid.tile([KP, XW - 4], F32)
                nc.vector.tensor_tensor(out=hx[:], in0=hxa[:],
                                        in1=x_t[:, 2:XW - 2], op=OP.add)

                for ch in range(2):
                    base = 1 + CHUNK * ch   # x-tile col of out col 0
                    hb = CHUNK * ch

                    pm = psum.tile([BAND, CHUNK], F32)
                    nc.tensor.matmul(out=pm[:], lhsT=w_box[:],
                                     rhs=hx[:, hb:hb + CHUNK],
                                     start=True, stop=True)
                    ps2 = psum.tile([BAND, CHUNK], F32)
                    for dx in range(3):
                        b2 = base - 1 + dx
                        nc.tensor.matmul(out=ps2[:], lhsT=w_box[:],
                                         rhs=sq[:, b2:b2 + CHUNK],
                                         start=(dx == 0), stop=(dx == 2))
                    ps3 = psum.tile([BAND, CHUNK], F32)
                    for dx in range(3):
                        b2 = base - 1 + dx
                        nc.tensor.matmul(out=ps3[:], lhsT=w_boxn[:],
                                         rhs=e_t[:, b2:b2 + CHUNK],
                                         start=(dx == 0), stop=(dx == 2))

                    m_s = small.tile([BAND, CHUNK], F32)
                    nc.scalar.activation(m_s[:], pm[:], AF.Copy)
                    m2 = small.tile([BAND, CHUNK], F32)
                    nc.scalar.activation(m2[:], pm[:], AF.Square)

                    dstack = dpool.tile([BAND, 9, CHUNK], F32)
                    xrows = (x_t, x_mid, x_dn)
                    for gi in range(9):
                        dy, dx = gi // 3 - 1, gi % 3 - 1
                        eng = nc.vector if (gi in DVE_SUBS or not use_gp) \
                            else nc.gpsimd
                        eng.tensor_tensor(
                            out=dstack[:, gi, :],
                            in0=xrows[dy + 1][0:BAND,
                                              base + dx:base + dx + CHUNK],
                            in1=m_s[:], op=OP.subtract)
                    hsum = small.tile([BAND, CHUNK], F32)
                    if use_reduce:
                        nc.vector.tensor_reduce(
                            out=hsum[:],
                            in_=dstack[:].rearrange("p g j -> p j g"),
                            axis=mybir.AxisListType.X, op=OP.add,
                            apply_absolute_value=True)
                    else:
                        for gi in range(9):
                            nc.scalar.activation(dstack[:, gi, :],
                                                 dstack[:, gi, :], AF.Abs)
                        nc.vector.tensor_tensor(out=hsum[:],
                                                in0=dstack[:, 0, :],
                                                in1=dstack[:, 1, :], op=OP.add)
                        for gi in range(2, 9):
                            nc.vector.tensor_tensor(out=hsum[:], in0=hsum[:],
                                                    in1=dstack[:, gi, :],
                                                    op=OP.add)

                    lnz = small.tile([BAND, CHUNK], F32)
                    nc.scalar.activation(lnz[:], hsum[:], AF.Ln,
                                         scale=1.0 / 9.0, bias=b_z[:])
                    hexp = small.tile([BAND, CHUNK], F32)
                    nc.scalar.activation(hexp[:], lnz[:], AF.Exp,
                                         scale=-1.0, bias=b_lnb[:])
                    o_hom = outp.tile([BAND, CHUNK], F32)
                    nc.vector.tensor_scalar_max(o_hom[:], hexp[:], 1e-4)

                    dsc = small.tile([BAND, CHUNK], F32)
                    if use_ttr:
                        scr = small.tile([BAND, 1], F32)
                        nc.vector.tensor_tensor_reduce(
                            out=dsc[:], in0=ps2[:], in1=m2[:],
                            scale=1e6 * BETA, scalar=0.0, op0=OP.subtract,
                            op1=OP.min, accum_out=scr[:])
                    else:
                        nc.vector.tensor_tensor(out=dsc[:], in0=ps2[:],
                                                in1=m2[:], op=OP.subtract)
                        nc.vector.tensor_scalar(out=dsc[:], in0=dsc[:],
                                                scalar1=1e6 * BETA,
                                                scalar2=None, op0=OP.mult)
                    o_con = outp.tile([BAND, CHUNK], F32)
                    nc.vector.tensor_scalar(out=o_con[:], in0=dsc[:],
                                            scalar1=8.0 * BETA / 9.0,
                                            scalar2=1e-4,
                                            op0=OP.min, op1=OP.max)
                    o_en = outp.tile([BAND, CHUNK], F32)
                    nc.vector.tensor_scalar(out=o_en[:], in0=ps2[:],
                                            scalar1=BETA, scalar2=1e-4,
                                            op0=OP.mult, op1=OP.max)
                    o_ent = outp.tile([BAND, CHUNK], F32)
                    nc.vector.tensor_scalar_max(o_ent[:], ps3[:], 1e-4)

                    for u2 in range(2):
                        s = g * UNITS + 2 * ch + u2
                        jj = UCOL * u2
                        for f, o_t in enumerate((o_con, o_en, o_ent, o_hom)):
                            nc.sync.dma_start(
                                out=out_d[s * 4 + f, r0:r0 + BAND, :],
                                in_=o_t[:, jj:jj + W])
    nc.compile()
    return nc


_CACHE = {}


def kernel(x: np.ndarray) -> np.ndarray:
    assert x.shape == (B, C, H, W) and x.dtype == np.float32
    if "nc" not in _CACHE:
        _CACHE["nc"] = _build()
    nc = _CACHE["nc"]
    w_box = _banded(1.0 / 9.0)
    w_boxn = _banded(-BETA / 9.0)
    in_maps = [{"x": np.ascontiguousarray(x[b]),
                "w_box": w_box, "w_boxn": w_boxn} for b in range(B)]
    res = run_bass_kernel_spmd(nc, in_maps, list(range(NCORES)))
    out = np.stack([res.results[b]["out"] for b in range(B)])
    return out.reshape(B, C * 4, H, W)


# revision 29
# speedup vs baseline: 48.4325x; 48.4325x over previous
"""Trainium2 Bass kernel for MedSegNetV2 GLCM-feature martingale.

Math (K=3 window, THETA=1, per pixel over zero-padded 3x3 neighborhood):
  m   = mean(win)                 (box9 / 9)
  M2  = mean(win^2)
  d   = M2 - m^2                  (biased variance, >= 0 up to fp error)
  contrast = d / max(1.125*d, 1e-6)        == min(8/9, 1e6*d)  exactly
  energy   = M2
  entropy  = -mean(t*ln t), t = max(win, 1e-6)
  homog    = 1 / (1 + mean|win - m| + 1e-6)
  out_f    = clip(max(f,1e-5)*exp(-.5), 1e-4, 1e4)
           == max(beta*f, 1e-4) for these value ranges (upper clip can't bind:
              energy<=~30, contrast<=8/9, |entropy|<=~10, homog<=1)

Sharding: pure data parallel, core k processes batch k (64 channel slices).
Layout per core: 16 groups of 4 slices; per group 2 row-bands of 112 rows;
x tile [114 part = 112 rows + 2 halo, 908 = 4*(224+2 pad cols) + 4 spare].
Vertical 3-sums via TensorE banded matmuls; horizontal 3-sums via shifted
col accesses (DVE adds for x, shifted-rhs accumulating matmuls for sq/e).
|win - m| for the 9 offsets: 9 tensor_tensor subs (DVE/GPSIMD split) into an
interleaved stack, then one DVE tensor_reduce(add, apply_absolute_value).
1/z computed as exp(ln(beta) - ln(z)) on ACT (no reciprocal needed).
"""

import math
from contextlib import ExitStack

import numpy as np

import concourse.bass as bass
import concourse.bacc as bacc
import concourse.tile as tile
from concourse import mybir
from concourse.bass_utils import run_bass_kernel_spmd

F32 = mybir.dt.float32
AF = mybir.ActivationFunctionType
OP = mybir.AluOpType

B, C, H, W = 8, 64, 224, 224
NCORES = 8
BETA = math.exp(-0.5)
LNBETA = -0.5

GROUPS = 16          # groups of 4 slices per core
UNITS = 4            # slices per group
UCOL = 226           # per-slice unit width: [pad, 224 data, pad]
XW = UNITS * UCOL + 4  # 908, even, includes 4 zero spare cols
BAND = 112           # output rows per band
KP = BAND + 2        # input partitions incl halo rows
CHUNK = 452          # output cols per chunk (= 2 units)

# which of the 9 (dy,dx) subtractions run on DVE vs GPSIMD
DVE_SUBS = (0, 2, 4, 6)


def _banded(val: float) -> np.ndarray:
    w = np.zeros((KP, BAND), dtype=np.float32)
    for p in range(BAND):
        for k in (p, p + 1, p + 2):
            w[k, p] = val
    return w


def _shift(dy: int) -> np.ndarray:
    # out(p, :) = x(partition p + 1 + dy, :)
    w = np.zeros((KP, BAND), dtype=np.float32)
    for p in range(BAND):
        w[p + 1 + dy, p] = 1.0
    return w


def _build(groups=GROUPS, num_devices=NCORES,
           use_reduce=True, use_ttr=False, use_gp=True, use_pefold=True,
           use_bf16=False):
    nslice = groups * UNITS
    nc = bacc.Bacc("TRN2", target_bir_lowering=False, debug=False,
                   num_devices=num_devices)
    x_in = nc.dram_tensor("x", [nslice, H, W], F32, kind="ExternalInput")
    w_box_d = nc.dram_tensor("w_box", [KP, BAND], F32, kind="ExternalInput")
    w_boxn_d = nc.dram_tensor("w_boxn", [KP, BAND], F32, kind="ExternalInput")
    w_boxm_d = nc.dram_tensor("w_boxm", [KP, BAND], F32, kind="ExternalInput")
    w_sm1_d = nc.dram_tensor("w_sm1", [KP, BAND], F32, kind="ExternalInput")
    w_sp1_d = nc.dram_tensor("w_sp1", [KP, BAND], F32, kind="ExternalInput")
    out_d = nc.dram_tensor("out", [nslice * 4, H, W], F32,
                           kind="ExternalOutput")

    with tile.TileContext(nc) as tc, ExitStack() as ctx:
        consts = ctx.enter_context(tc.tile_pool(name="consts", bufs=1))
        io = ctx.enter_context(tc.tile_pool(name="io", bufs=2))
        mid = ctx.enter_context(tc.tile_pool(name="mid", bufs=2))
        dpool = ctx.enter_context(tc.tile_pool(name="dpool", bufs=2))
        small = ctx.enter_context(tc.tile_pool(name="small", bufs=3))
        outp = ctx.enter_context(tc.tile_pool(name="outp", bufs=3))
        psum = ctx.enter_context(tc.tile_pool(name="psum", bufs=2, space="PSUM"))
        psumd = ctx.enter_context(tc.tile_pool(name="psumd", bufs=2,
                                               space="PSUM"))

        w_box = consts.tile([KP, BAND], F32)
        w_boxn = consts.tile([KP, BAND], F32)
        w_boxm = consts.tile([KP, BAND], F32)
        w_sm1 = consts.tile([KP, BAND], F32)
        w_sp1 = consts.tile([KP, BAND], F32)
        nc.sync.dma_start(out=w_box[:], in_=w_box_d[:])
        nc.sync.dma_start(out=w_boxn[:], in_=w_boxn_d[:])
        nc.sync.dma_start(out=w_boxm[:], in_=w_boxm_d[:])
        nc.sync.dma_start(out=w_sm1[:], in_=w_sm1_d[:])
        nc.sync.dma_start(out=w_sp1[:], in_=w_sp1_d[:])
        b_z = consts.tile([BAND, 1], F32)
        nc.vector.memset(b_z[:], 1.0 + 1e-6)
        b_lnb = consts.tile([BAND, 1], F32)
        nc.vector.memset(b_lnb[:], LNBETA)

        for g in range(groups):
            for half in range(2):
                r0 = half * BAND
                x_t = io.tile([KP, XW], F32)
                nc.gpsimd.memset(x_t[:], 0.0)
                for u in range(UNITS):
                    s = g * UNITS + u
                    c0 = UCOL * u + 1
                    if half == 0:
                        nc.sync.dma_start(out=x_t[1:KP, c0:c0 + W],
                                          in_=x_in[s, 0:KP - 1, :])
                    else:
                        nc.sync.dma_start(out=x_t[0:KP - 1, c0:c0 + W],
                                          in_=x_in[s, r0 - 1:H, :])

                # engines can't read SBUF at partition offsets 1/2 — make
                # row-shifted copies via DMA (partition shifts legal there)
                x_mid = io.tile([BAND, XW], F32)
                nc.sync.dma_start(out=x_mid[:], in_=x_t[1:1 + BAND, :])
                x_dn = io.tile([BAND, XW], F32)
                nc.sync.dma_start(out=x_dn[:], in_=x_t[2:2 + BAND, :])

                gp = nc.gpsimd if use_gp else nc.vector
                tcl = mid.tile([KP, XW], F32)
                gp.tensor_scalar_max(tcl[:], x_t[:], 1e-6)
                lnt = mid.tile([KP, XW], F32)
                nc.scalar.activation(lnt[:], tcl[:], AF.Ln)
                e_t = mid.tile([KP, XW], F32)
                gp.tensor_tensor(out=e_t[:], in0=tcl[:], in1=lnt[:],
                                 op=OP.mult)
                sq = mid.tile([KP, XW], F32)
                nc.scalar.activation(sq[:], x_t[:], AF.Square)
                hxa = mid.tile([KP, XW - 4], F32)
                nc.vector.tensor_tensor(out=hxa[:], in0=x_t[:, 0:XW - 4],
                                        in1=x_t[:, 1:XW - 3], op=OP.add)
                hx = mid.tile([KP, XW - 4], F32)
                gp.tensor_tensor(out=hx[:], in0=hxa[:],
                                 in1=x_t[:, 2:XW - 2], op=OP.add)

                for ch in range(2):
                    base = 1 + CHUNK * ch   # x-tile col of out col 0
                    hb = CHUNK * ch

                    pm = psum.tile([BAND, CHUNK], F32)
                    nc.tensor.matmul(out=pm[:], lhsT=w_box[:],
                                     rhs=hx[:, hb:hb + CHUNK],
                                     start=True, stop=True)
                    ps2 = psum.tile([BAND, CHUNK], F32)
                    for dx in range(3):
                        b2 = base - 1 + dx
                        nc.tensor.matmul(out=ps2[:], lhsT=w_box[:],
                                         rhs=sq[:, b2:b2 + CHUNK],
                                         start=(dx == 0), stop=(dx == 2))
                    ps3 = psum.tile([BAND, CHUNK], F32)
                    for dx in range(3):
                        b2 = base - 1 + dx
                        nc.tensor.matmul(out=ps3[:], lhsT=w_boxn[:],
                                         rhs=e_t[:, b2:b2 + CHUNK],
                                         start=(dx == 0), stop=(dx == 2))

                    m_s = small.tile([BAND, CHUNK], F32)
                    nc.scalar.activation(m_s[:], pm[:], AF.Copy)
                    m2 = small.tile([BAND, CHUNK], F32)
                    nc.scalar.activation(m2[:], pm[:], AF.Square)

                    # bf16 mode: g innermost (+1 zero pad lane) so the
                    # abs-reduce reads contiguous 16-bit pairs (2x mode)
                    BF = mybir.dt.bfloat16
                    if use_bf16:
                        dstack = dpool.tile([BAND, CHUNK, 10], BF)
                        nc.gpsimd.memset(dstack[:, :, 9], 0.0)
                        dplane = lambda gi: dstack[:, :, gi]
                    else:
                        dstack = dpool.tile([BAND, 9, CHUNK], F32)
                        dplane = lambda gi: dstack[:, gi, :]
                    xrows = (x_t, x_mid, x_dn)
                    for gi in range(9):
                        dy, dx = gi // 3 - 1, gi % 3 - 1
                        if use_pefold and dy != 0:
                            w_sh = w_sm1 if dy < 0 else w_sp1
                            dps = psumd.tile([BAND, CHUNK], F32)
                            nc.tensor.matmul(out=dps[:], lhsT=w_sh[:],
                                             rhs=x_t[:, base + dx:
                                                     base + dx + CHUNK],
                                             start=True, stop=False)
                            nc.tensor.matmul(out=dps[:], lhsT=w_boxm[:],
                                             rhs=hx[:, hb:hb + CHUNK],
                                             start=False, stop=True)
                            nc.scalar.activation(dplane(gi), dps[:],
                                                 AF.Abs)
                        else:
                            eng = nc.vector if (gi in DVE_SUBS or not use_gp) \
                                else nc.gpsimd
                            eng.tensor_tensor(
                                out=dplane(gi),
                                in0=xrows[dy + 1][0:BAND,
                                                  base + dx:base + dx + CHUNK],
                                in1=m_s[:], op=OP.subtract)
                    hsum = small.tile([BAND, CHUNK], F32)
                    if use_reduce:
                        red_in = dstack[:] if use_bf16 else \
                            dstack[:].rearrange("p g j -> p j g")
                        nc.vector.tensor_reduce(
                            out=hsum[:], in_=red_in,
                            axis=mybir.AxisListType.X, op=OP.add,
                            apply_absolute_value=True)
                    else:
                        assert not use_bf16
                        for gi in range(9):
                            nc.scalar.activation(dstack[:, gi, :],
                                                 dstack[:, gi, :], AF.Abs)
                        nc.vector.tensor_tensor(out=hsum[:],
                                                in0=dstack[:, 0, :],
                                                in1=dstack[:, 1, :], op=OP.add)
                        for gi in range(2, 9):
                            nc.vector.tensor_tensor(out=hsum[:], in0=hsum[:],
                                                    in1=dstack[:, gi, :],
                                                    op=OP.add)

                    lnz = small.tile([BAND, CHUNK], F32)
                    nc.scalar.activation(lnz[:], hsum[:], AF.Ln,
                                         scale=1.0 / 9.0, bias=b_z[:])
                    hexp = small.tile([BAND, CHUNK], F32)
                    nc.scalar.activation(hexp[:], lnz[:], AF.Exp,
                                         scale=-1.0, bias=b_lnb[:])
                    o_hom = outp.tile([BAND, CHUNK], F32)
                    nc.vector.tensor_scalar_max(o_hom[:], hexp[:], 1e-4)

                    dsc = small.tile([BAND, CHUNK], F32)
                    if use_ttr:
                        scr = small.tile([BAND, 1], F32)
                        nc.vector.tensor_tensor_reduce(
                            out=dsc[:], in0=ps2[:], in1=m2[:],
                            scale=1e6 * BETA, scalar=0.0, op0=OP.subtract,
                            op1=OP.min, accum_out=scr[:])
                    else:
                        nc.vector.tensor_tensor(out=dsc[:], in0=ps2[:],
                                                in1=m2[:], op=OP.subtract)
                        nc.vector.tensor_scalar(out=dsc[:], in0=dsc[:],
                                                scalar1=1e6 * BETA,
                                                scalar2=None, op0=OP.mult)
                    o_con = outp.tile([BAND, CHUNK], F32)
                    nc.vector.tensor_scalar(out=o_con[:], in0=dsc[:],
                                            scalar1=8.0 * BETA / 9.0,
                                            scalar2=1e-4,
                                            op0=OP.min, op1=OP.max)
                    o_en = outp.tile([BAND, CHUNK], F32)
                    nc.vector.tensor_scalar(out=o_en[:], in0=ps2[:],
                                            scalar1=BETA, scalar2=1e-4,
                                            op0=OP.mult, op1=OP.max)
                    o_ent = outp.tile([BAND, CHUNK], F32)
                    nc.vector.tensor_scalar_max(o_ent[:], ps3[:], 1e-4)

                    for u2 in range(2):
                        s = g * UNITS + 2 * ch + u2
                        jj = UCOL * u2
                        for f, o_t in enumerate((o_con, o_en, o_ent, o_hom)):
                            nc.sync.dma_start(
                                out=out_d[s * 4 + f, r0:r0 + BAND, :],
                                in_=o_t[:, jj:jj + W])
    nc.compile()
    return nc


_CACHE = {}


def _weights() -> dict:
    return {"w_box": _banded(1.0 / 9.0), "w_boxn": _banded(-BETA / 9.0),
            "w_boxm": _banded(-1.0 / 9.0), "w_sm1": _shift(-1),
            "w_sp1": _shift(+1)}


def kernel(x: np.ndarray) -> np.ndarray:
    assert x.shape == (B, C, H, W) and x.dtype == np.float32
    if "nc" not in _CACHE:
        _CACHE["nc"] = _build()
    nc = _CACHE["nc"]
    in_maps = [{"x": np.ascontiguousarray(x[b]), **_weights()}
               for b in range(B)]
    res = run_bass_kernel_spmd(nc, in_maps, list(range(NCORES)))
    out = np.stack([res.results[b]["out"] for b in range(B)])
    return out.reshape(B, C * 4, H, W)
